# revision 34
# baseline (speedup 1.0000x reference)
"""Kimi-K2.5 tensorized MoE kernel for 8 TRN2 NeuronCores.

Sharding: expert-parallel. Core c owns routed experts [4c, 4c+4) and rows
[128c, 128(c+1)) of the shared-expert intermediate. The router runs
replicated on every core in fp32 (selection must match the reference
exactly), pipelined per 512-token chunk with the expert matmuls; the
top-k logic is batched across the chunk's four 128-token tiles with
multi-dim access patterns so it stays off the PE critical path.
Expert/shared matmuls run in bf16 with fp32 PSUM accumulation.

Per-core partial outputs are reduced on-device with bf16 ReduceScatters,
two per 512-token chunk (h rows [0,512) and [512,1024) separately so the
first collective can start mid-chunk). With the h-split, core c ends up
holding rows [64c, 64c+64) of the first h-half and rows [512+64c,
512+64c+64) of the second for every token column; the host reassembles
accordingly and transposes back to [B, S, H] fp32.
"""

import os
import sys

sys.path.insert(0, "/opt/trn_rl_repo")

import numpy as np
import ml_dtypes

from concourse import bass, bacc, mybir, tile
from concourse.bass_utils import run_bass_kernel_spmd

F32 = mybir.dt.float32
BF16 = mybir.dt.bfloat16
AF = mybir.ActivationFunctionType
ALU = mybir.AluOpType
AX = mybir.AxisListType

B, S, H = 2, 1024, 1024
T = B * S                 # 2048 tokens
I = 512                   # moe intermediate
E = 32                    # routed experts
TOP_K = 4
N_GROUP = 4
GRP = E // N_GROUP        # 8 experts per group
TOPK_GROUP = 2
SCALE = 2.5
SH_I = 1024               # shared intermediate (2 * I)
NCORES = 8
E_LOC = E // NCORES       # 4 experts per core
SH_LOC = SH_I // NCORES   # 128 shared-intermediate rows per core

P = 128
TC = 512                  # t-chunk (moving free dim)
NC_ = T // TC             # 4 t-chunks
NTT = TC // P             # 4 t-tiles per chunk
NH = H // P               # 8 h-tiles
NI = I // P               # 4 i-tiles per expert
# ReduceScatter piece boundaries (in h-tiles of 128 rows) per chunk:
# two even halves, so the first can start mid-way through the down phase.
PIECES = {c: [(0, 4), (4, 8)] for c in range(4)}


def _build(trace: bool = False):
    nc = bacc.Bacc("TRN2", target_bir_lowering=False, debug=False,
                   num_devices=NCORES)

    # ---- kernel I/O. All inputs are pre-packed on the host so every
    # DMA is a straight 2D copy with long contiguous runs per partition.
    tokf = nc.dram_tensor("tokf", [P, NC_ * NH * TC], F32,
                          kind="ExternalInput")
    tokb = nc.dram_tensor("tokb", [P, NC_ * NH * TC], BF16,
                          kind="ExternalInput")
    rwT = nc.dram_tensor("rwT", [P, NH * E], F32, kind="ExternalInput")
    rbias = nc.dram_tensor("rbias", [1, E], F32, kind="ExternalInput")
    ident = nc.dram_tensor("ident", [P, P], F32, kind="ExternalInput")
    selb = nc.dram_tensor("selb", [E, E_LOC * P], BF16,
                          kind="ExternalInput")
    gwT = nc.dram_tensor("gwT", [P, E_LOC * NH * I], BF16,
                         kind="ExternalInput")
    uwT = nc.dram_tensor("uwT", [P, E_LOC * NH * I], BF16,
                         kind="ExternalInput")
    dwT = nc.dram_tensor("dwT", [P, E_LOC * NI * H], BF16,
                         kind="ExternalInput")
    sgwT = nc.dram_tensor("sgwT", [P, NH * SH_LOC], BF16,
                          kind="ExternalInput")
    suwT = nc.dram_tensor("suwT", [P, NH * SH_LOC], BF16,
                          kind="ExternalInput")
    sdwT = nc.dram_tensor("sdwT", [SH_LOC, H], BF16, kind="ExternalInput")
    out_shard = nc.dram_tensor("out_shard", [P, T], BF16,
                               kind="ExternalOutput")

    rg = [list(range(NCORES))]

    with tile.TileContext(nc) as tc:
        with (
            tc.tile_pool(name="resident", bufs=1) as rp,
            tc.tile_pool(name="router", bufs=1) as rr,
            tc.tile_pool(name="work", bufs=2) as xp,
            tc.tile_pool(name="hid", bufs=1) as hp,
            tc.tile_pool(name="psum", bufs=2, space="PSUM") as ps,
            tc.tile_pool(name="dram", bufs=1, space="DRAM") as dp,
        ):
            # ---------- DMA priority order ----------
            # tiny consts first
            ident_sb = rp.tile([P, P], F32, tag="ident")
            nc.sync.dma_start(ident_sb[:], ident[:, :])
            rbias_sb = rp.tile([1, E], F32, tag="rbias")
            nc.sync.dma_start(rbias_sb[:], rbias[:, :])
            selb_sb = rp.tile([E, E_LOC, P], BF16, tag="selb")
            nc.sync.dma_start(selb_sb[:].rearrange("e l p -> e (l p)"),
                              selb[:, :])
            rw_sb = rp.tile([P, NH, E], F32, tag="rw")
            nc.sync.dma_start(rw_sb[:].rearrange("p a e -> p (a e)"),
                              rwT[:, :])

            # chunk-0 tokens. tokb (bf16, read all chunk long) is resident
            # per chunk; tokf (fp32, router-only) streams through a
            # 2-buffer ring.
            CW = NH * TC
            tokf_sb, tokb_sb = {}, {}

            def load_tokf(c):
                t_ = rp.tile([P, NH, TC], F32, tag="tokf", bufs=2,
                             name=f"tokf{c}")
                nc.sync.dma_start(t_[:].rearrange("p a t -> p (a t)"),
                                  tokf[:, c * CW:(c + 1) * CW])
                tokf_sb[c] = t_

            def load_tokb(c):
                t_ = rp.tile([P, NH, TC], BF16, tag="tokb", bufs=3,
                             name=f"tokb{c}")
                nc.sync.dma_start(t_[:].rearrange("p a t -> p (a t)"),
                                  tokb[:, c * CW:(c + 1) * CW])
                tokb_sb[c] = t_

            load_tokf(0)
            load_tokb(0)

            # shared-expert weights first (small, lets the PE start on the
            # shared matmuls while the bigger routed weights stream in)
            sgw_sb = rp.tile([P, NH, SH_LOC], BF16, tag="sgw")
            nc.sync.dma_start(sgw_sb[:].rearrange("p a s -> p (a s)"),
                              sgwT[:, :])
            suw_sb = rp.tile([P, NH, SH_LOC], BF16, tag="suw")
            nc.sync.dma_start(suw_sb[:].rearrange("p a s -> p (a s)"),
                              suwT[:, :])

            # gate/up weights, expert-major so expert 0 lands first
            EW = NH * I
            gw_sb, uw_sb = [], []
            for el in range(E_LOC):
                g_ = rp.tile([P, NH, I], BF16, tag=f"gw{el}")
                nc.sync.dma_start(g_[:].rearrange("p a i -> p (a i)"),
                                  gwT[:, el * EW:(el + 1) * EW])
                gw_sb.append(g_)
                u_ = rp.tile([P, NH, I], BF16, tag=f"uw{el}")
                nc.sync.dma_start(u_[:].rearrange("p a i -> p (a i)"),
                                  uwT[:, el * EW:(el + 1) * EW])
                uw_sb.append(u_)

            # chunk-1 tokens, then down weights (chunks 2-3 tokens are
            # prefetched from inside the chunk loop)
            load_tokf(1)
            load_tokb(1)
            dw_sb = rp.tile([P, E_LOC, NI, H], BF16, tag="dw")
            nc.sync.dma_start(dw_sb[:].rearrange("p l it h -> p (l it h)"),
                              dwT[:, :])
            sdw_sb = rp.tile([SH_LOC, H], BF16, tag="sdw")
            nc.sync.dma_start(sdw_sb[:], sdwT[:, :])

            # ---------- router bias broadcast [P, E] ----------
            ones = rp.tile([1, P], F32, tag="ones")
            nc.vector.memset(ones[:], 1.0)
            bias_ps = ps.tile([P, E], F32, tag="misc")
            nc.tensor.matmul(bias_ps[:], ones[:], rbias_sb[:],
                             start=True, stop=True)
            bias_b = rp.tile([P, E], F32, tag="bias_b")
            nc.scalar.copy(bias_b[:], bias_ps[:])

            # ---------- pipelined chunk loop ----------
            for c in range(NC_):
                tsl = slice(c * TC, (c + 1) * TC)
                if c + 2 < NC_:
                    load_tokf(c + 2)
                    load_tokb(c + 2)
                tkf, tkb = tokf_sb[c], tokb_sb[c]

                # --- router scores: fp32 matmul + sigmoid per t-tile ---
                scores = rr.tile([P, NTT, E], F32, tag="scores")
                for tt in range(NTT):
                    sc_ps = ps.tile([P, E], F32, tag="misc")
                    for ht in range(NH):
                        nc.tensor.matmul(
                            sc_ps[:], tkf[:, ht, tt * P:(tt + 1) * P],
                            rw_sb[:, ht, :],
                            start=(ht == 0), stop=(ht == NH - 1))
                    nc.scalar.activation(scores[:, tt, :], sc_ps[:],
                                         AF.Sigmoid)

                # --- gate/up. silu is computed as g * sigmoid(g) * u so
                # the Act engine only ever uses the sigmoid table (a
                # Sigmoid<->Silu swap costs a 1.3us table load per switch).
                # The sigmoid (Act) + g*u (DVE) drain the PSUM banks right
                # behind each group without waiting on the router; the
                # routing weight is multiplied in once wb is ready. ---
                hid = {}

                def emit_swiglu(gp, up, h_):
                    # silu(g)*u = g*sigmoid(g)*u with each DVE op reading
                    # at most one PSUM operand
                    sg_t = xp.tile([P, TC], F32, tag="sg", name="sg_t")
                    nc.scalar.activation(sg_t[:], gp[:], AF.Sigmoid)
                    gu_t = xp.tile([P, TC], F32, tag="gu", name="gu_t",
                                   bufs=1)
                    nc.vector.tensor_tensor(gu_t[:], up[:], sg_t[:],
                                            op=ALU.mult)
                    nc.vector.tensor_tensor(h_[:], gu_t[:], gp[:],
                                            op=ALU.mult)

                def emit_gate_up(el):
                    for it in range(NI):
                        isl = slice(it * P, (it + 1) * P)
                        gp = ps.tile([P, TC], F32, tag="g_ps", name="gp")
                        up = ps.tile([P, TC], F32, tag="u_ps", name="up")
                        for ht in range(NH):
                            nc.tensor.matmul(gp[:], gw_sb[el][:, ht, isl],
                                             tkb[:, ht, :],
                                             start=(ht == 0),
                                             stop=(ht == NH - 1))
                        for ht in range(NH):
                            nc.tensor.matmul(up[:], uw_sb[el][:, ht, isl],
                                             tkb[:, ht, :],
                                             start=(ht == 0),
                                             stop=(ht == NH - 1))
                        h_ = hp.tile([P, TC], BF16, tag=f"hid{el}_{it}",
                                     name=f"hid{el}_{it}")
                        emit_swiglu(gp, up, h_)
                        hid[(el, it)] = h_

                # shared expert first: its weights land earliest and its
                # hidden state has no routing-weight dependency at all
                sg_ps = ps.tile([P, TC], F32, tag="g_ps")
                su_ps = ps.tile([P, TC], F32, tag="u_ps")
                for ht in range(NH):
                    nc.tensor.matmul(sg_ps[:], sgw_sb[:, ht, :],
                                     tkb[:, ht, :],
                                     start=(ht == 0), stop=(ht == NH - 1))
                for ht in range(NH):
                    nc.tensor.matmul(su_ps[:], suw_sb[:, ht, :],
                                     tkb[:, ht, :],
                                     start=(ht == 0), stop=(ht == NH - 1))
                sh_hid = hp.tile([P, TC], BF16, tag="sh_hid")
                emit_swiglu(sg_ps, su_ps, sh_hid)

                emit_gate_up(0)

                # --- batched top-k router chain (DVE) ---
                def r3(t_):
                    return t_[:]
                def r4(t_):
                    return t_[:].rearrange("p a (g e) -> p a g e", e=GRP)

                sfc = rr.tile([P, NTT, E], F32, tag="sfc")
                nc.vector.tensor_tensor(
                    sfc[:], scores[:],
                    bias_b[:].unsqueeze(1).broadcast_to([P, NTT, E]),
                    op=ALU.add)
                m1 = rr.tile([P, NTT, N_GROUP], F32, tag="m1")
                nc.vector.tensor_reduce(m1[:], r4(sfc), axis=AX.X,
                                        op=ALU.max)
                eq = rr.tile([P, NTT, E], F32, tag="eq")
                nc.vector.tensor_tensor(
                    r4(eq), r4(sfc),
                    m1[:].unsqueeze(3).broadcast_to([P, NTT, N_GROUP, GRP]),
                    op=ALU.is_equal)
                tmp = rr.tile([P, NTT, E], F32, tag="tmp")
                nc.vector.tensor_scalar_mul(tmp[:], eq[:], 1e30)
                wo = rr.tile([P, NTT, E], F32, tag="wo")
                nc.vector.tensor_tensor(wo[:], sfc[:], tmp[:],
                                        op=ALU.subtract)
                m2 = rr.tile([P, NTT, N_GROUP], F32, tag="m2")
                nc.vector.tensor_reduce(m2[:], r4(wo), axis=AX.X, op=ALU.max)
                gs = rr.tile([P, NTT, N_GROUP], F32, tag="gs")
                nc.vector.tensor_tensor(gs[:], m1[:], m2[:], op=ALU.add)
                gm1 = rr.tile([P, NTT], F32, tag="gm1")
                nc.vector.tensor_reduce(gm1[:], gs[:], axis=AX.X, op=ALU.max)
                eqg = rr.tile([P, NTT, N_GROUP], F32, tag="eqg")
                nc.vector.tensor_tensor(
                    eqg[:], gs[:],
                    gm1[:].unsqueeze(2).broadcast_to([P, NTT, N_GROUP]),
                    op=ALU.is_equal)
                tmpg = rr.tile([P, NTT, N_GROUP], F32, tag="tmpg")
                nc.vector.tensor_scalar_mul(tmpg[:], eqg[:], 1e30)
                gs2 = rr.tile([P, NTT, N_GROUP], F32, tag="gs2")
                nc.vector.tensor_tensor(gs2[:], gs[:], tmpg[:],
                                        op=ALU.subtract)
                gm2 = rr.tile([P, NTT], F32, tag="gm2")
                nc.vector.tensor_reduce(gm2[:], gs2[:], axis=AX.X,
                                        op=ALU.max)
                gmask = rr.tile([P, NTT, N_GROUP], F32, tag="gmask")
                nc.vector.tensor_tensor(
                    gmask[:], gs[:],
                    gm2[:].unsqueeze(2).broadcast_to([P, NTT, N_GROUP]),
                    op=ALU.is_ge)
                masked = rr.tile([P, NTT, E], F32, tag="masked")
                nc.vector.tensor_tensor(
                    r4(masked), r4(sfc),
                    gmask[:].unsqueeze(3).broadcast_to(
                        [P, NTT, N_GROUP, GRP]),
                    op=ALU.mult)
                mx = rr.tile([P, NTT * 8], F32, tag="mx")
                for tt in range(NTT):
                    nc.vector.max(mx[:, tt * 8:(tt + 1) * 8],
                                  masked[:, tt, :])
                m4b = (mx[:].rearrange("p (a k) -> p a k", k=8)[:, :, 3:4]
                       .broadcast_to([P, NTT, E]))
                sel = rr.tile([P, NTT, E], F32, tag="sel")
                nc.vector.tensor_tensor(sel[:], masked[:], m4b,
                                        op=ALU.is_ge)
                wun = rr.tile([P, NTT, E], F32, tag="wun")
                nc.vector.tensor_tensor(wun[:], scores[:], sel[:],
                                        op=ALU.mult)
                den = rr.tile([P, NTT], F32, tag="den")
                nc.vector.tensor_reduce(den[:], wun[:], axis=AX.X,
                                        op=ALU.add)
                nc.vector.tensor_scalar_add(den[:], den[:], 1e-20)
                rec = rr.tile([P, NTT], F32, tag="rec")
                nc.vector.reciprocal(rec[:], den[:])
                nc.vector.tensor_scalar_mul(rec[:], rec[:], SCALE)
                wfin = rr.tile([P, NTT, E], F32, tag="wfin")
                nc.vector.tensor_tensor(
                    wfin[:], wun[:],
                    rec[:].unsqueeze(2).broadcast_to([P, NTT, E]),
                    op=ALU.mult)

                # --- wt = wfin^T (PE transpose) then per-expert broadcast ---
                wt_sb = rr.tile([E, NTT, P], BF16, tag="wt")
                for tt in range(NTT):
                    wt_ps = ps.tile([E, P], F32, tag="misc")
                    nc.tensor.transpose(wt_ps[:], wfin[:, tt, :],
                                        ident_sb[:])
                    nc.scalar.copy(wt_sb[:, tt, :], wt_ps[:])
                wb_sb = {}
                for el in range(E_LOC):
                    wb_ps = ps.tile([P, TC], F32, tag="misc")
                    nc.tensor.matmul(
                        wb_ps[:], selb_sb[:, el, :],
                        wt_sb[:].rearrange("e a t -> e (a t)"),
                        start=True, stop=True)
                    w_ = xp.tile([P, TC], F32, tag="wb", bufs=4)
                    nc.scalar.copy(w_[:], wb_ps[:])
                    wb_sb[el] = w_

                # --- experts 1-3 gate/up ---
                for el in range(1, E_LOC):
                    emit_gate_up(el)

                # --- scale hid by routing weights (in place, DVE) ---
                for el in range(E_LOC):
                    for it in range(NI):
                        h_ = hid[(el, it)]
                        nc.vector.tensor_tensor(h_[:], h_[:],
                                                wb_sb[el][:], op=ALU.mult)

                # --- down matmuls; bf16 partials to DRAM; two RS pieces
                # per chunk. The last chunk uses an uneven 6+2 split so
                # the only fully exposed collective is a small one. ---
                pieces = PIECES[c]
                cc_in = [dp.tile([(b - a) * P, TC], BF16,
                                 tag=f"cc_in{c}_{k}", name=f"cc_in{c}_{k}")
                         for k, (a, b) in enumerate(pieces)]
                for ht in range(NH):
                    hsl = slice(ht * P, (ht + 1) * P)
                    d_ps = ps.tile([P, TC], F32, tag="d_ps")
                    k = 0
                    for el in range(E_LOC):
                        for it in range(NI):
                            nc.tensor.matmul(d_ps[:],
                                             dw_sb[:, el, it, hsl],
                                             hid[(el, it)][:],
                                             start=(k == 0), stop=False)
                            k += 1
                    nc.tensor.matmul(d_ps[:], sdw_sb[:, hsl], sh_hid[:],
                                     start=False, stop=True)
                    o_sb = xp.tile([P, TC], BF16, tag="o_sb")
                    nc.vector.tensor_copy(o_sb[:], d_ps[:])
                    piece = next(k for k, (a, b) in enumerate(pieces)
                                 if a <= ht < b)
                    a, b = pieces[piece]
                    nc.sync.dma_start(cc_in[piece][(ht - a) * P:
                                                   (ht - a + 1) * P, :],
                                      o_sb[:])
                    if ht == b - 1:
                        rows = (b - a) * P // NCORES
                        cc_out = dp.tile([rows, TC], BF16,
                                         tag=f"cc_out{c}_{piece}",
                                         name=f"cc_out{c}_{piece}")
                        nc.gpsimd.collective_compute(
                            "ReduceScatter", ALU.add, replica_groups=rg,
                            ins=[cc_in[piece].opt()], outs=[cc_out.opt()],
                        )
                        nc.gpsimd.dma_start(
                            out_shard[a * P // NCORES:
                                      a * P // NCORES + rows, tsl],
                            cc_out[:])

    nc.compile()
    return nc


def _pack_rows(a):
    """[X*128, Y] row-major -> [128, X*Y] with per-partition layout (X, Y)."""
    X = a.shape[0] // P
    return np.ascontiguousarray(
        a.reshape(X, P, -1).transpose(1, 0, 2).reshape(P, -1))


def _prep_inputs(hidden_states, router_weight, router_bias, gate_w, up_w,
                 down_w, shared_gate_w, shared_up_w, shared_down_w):
    bf = ml_dtypes.bfloat16
    tokens = np.ascontiguousarray(
        np.asarray(hidden_states, dtype=np.float32).reshape(T, H))
    tokf = np.ascontiguousarray(tokens.T)                       # [H, T] f32
    # pack tokens chunk-major: [128, (chunk, h-tile, t)]
    tokf_p = np.ascontiguousarray(
        tokf.reshape(NH, P, NC_, TC).transpose(1, 2, 0, 3).reshape(P, -1))
    tokb_p = tokf_p.astype(bf)
    rw_p = _pack_rows(np.ascontiguousarray(
        np.asarray(router_weight, dtype=np.float32).T))         # [H, E]
    rbias = np.asarray(router_bias, dtype=np.float32).reshape(1, E)
    ident = np.eye(P, dtype=np.float32)
    gwT = np.ascontiguousarray(
        np.asarray(gate_w, dtype=np.float32).transpose(0, 2, 1)).astype(bf)
    uwT = np.ascontiguousarray(
        np.asarray(up_w, dtype=np.float32).transpose(0, 2, 1)).astype(bf)
    dwT = np.ascontiguousarray(
        np.asarray(down_w, dtype=np.float32).transpose(0, 2, 1)).astype(bf)
    sgwT = np.ascontiguousarray(
        np.asarray(shared_gate_w, dtype=np.float32).T)          # [H, SH_I]
    suwT = np.ascontiguousarray(
        np.asarray(shared_up_w, dtype=np.float32).T)
    sdwT = np.ascontiguousarray(
        np.asarray(shared_down_w, dtype=np.float32).T)          # [SH_I, H]

    in_maps = []
    for c in range(NCORES):
        esl = slice(c * E_LOC, (c + 1) * E_LOC)
        ssl = slice(c * SH_LOC, (c + 1) * SH_LOC)
        sel = np.zeros((E_LOC, E, P), dtype=np.float32)
        for el in range(E_LOC):
            sel[el, c * E_LOC + el, :] = 1.0
        # gw/uw: [128, (el, h-tile, i)]
        gw_loc = gwT[esl]            # [E_LOC, H, I]
        gw_p = np.ascontiguousarray(
            gw_loc.reshape(E_LOC, NH, P, I).transpose(2, 0, 1, 3)
            .reshape(P, -1))
        uw_loc = uwT[esl]
        uw_p = np.ascontiguousarray(
            uw_loc.reshape(E_LOC, NH, P, I).transpose(2, 0, 1, 3)
            .reshape(P, -1))
        # dw: [128, (el, i-tile, h)]
        dw_loc = dwT[esl]            # [E_LOC, I, H]
        dw_p = np.ascontiguousarray(
            dw_loc.reshape(E_LOC, NI, P, H).transpose(2, 0, 1, 3)
            .reshape(P, -1))
        in_maps.append({
            "tokf": tokf_p,
            "tokb": tokb_p,
            "rwT": rw_p,
            "rbias": rbias,
            "ident": ident,
            "selb": np.ascontiguousarray(
                sel.transpose(1, 0, 2).reshape(E, -1)).astype(bf),
            "gwT": gw_p,
            "uwT": uw_p,
            "dwT": dw_p,
            "sgwT": _pack_rows(np.ascontiguousarray(sgwT[:, ssl])
                               .astype(bf)),
            "suwT": _pack_rows(np.ascontiguousarray(suwT[:, ssl])
                               .astype(bf)),
            "sdwT": np.ascontiguousarray(sdwT[ssl, :]).astype(bf),
        })
    return in_maps


def run_on_device(inputs: dict, trace: bool = False, tmpdir: str | None = None):
    in_maps = _prep_inputs(**inputs)
    nc = _build(trace=trace)
    res = run_bass_kernel_spmd(nc, in_maps, list(range(NCORES)), trace=trace,
                               tmpdir=tmpdir)
    # Reassemble: for RS piece (a, b) of chunk ck, core c's shard rows
    # [16a, 16a + 16(b-a)) hold global h rows [128a + 16(b-a)c, ...).
    outT = np.empty((H, T), dtype=np.float32)
    for c in range(NCORES):
        sh = np.asarray(res.results[c]["out_shard"], dtype=np.float32)
        for ck in range(NC_):
            csl = slice(ck * TC, (ck + 1) * TC)
            for (a, b) in PIECES[ck]:
                w = (b - a) * P // NCORES
                outT[a * P + c * w:a * P + (c + 1) * w, csl] = \
                    sh[a * P // NCORES:a * P // NCORES + w, csl]
    out = np.ascontiguousarray(outT.T).reshape(B, S, H).astype(np.float32)
    return out, res


def kernel(**inputs) -> np.ndarray:
    out, _ = run_on_device(inputs, trace=False)
    return out


# revision 36
# speedup vs baseline: 1.0066x; 1.0066x over previous
"""Kimi-K2.5 tensorized MoE kernel for 8 TRN2 NeuronCores.

Sharding: expert-parallel. Core c owns routed experts [4c, 4c+4) and rows
[128c, 128(c+1)) of the shared-expert intermediate. The router runs
replicated on every core in fp32 (selection must match the reference
exactly), pipelined per 512-token chunk with the expert matmuls; the
top-k logic is batched across the chunk's four 128-token tiles with
multi-dim access patterns so it stays off the PE critical path.
Expert/shared matmuls run in bf16 with fp32 PSUM accumulation.

Per-core partial outputs are reduced on-device with bf16 ReduceScatters,
two per 512-token chunk (h rows [0,512) and [512,1024) separately so the
first collective can start mid-chunk). With the h-split, core c ends up
holding rows [64c, 64c+64) of the first h-half and rows [512+64c,
512+64c+64) of the second for every token column; the host reassembles
accordingly and transposes back to [B, S, H] fp32.
"""

import os
import sys

sys.path.insert(0, "/opt/trn_rl_repo")

import numpy as np
import ml_dtypes

from concourse import bass, bacc, mybir, tile
from concourse.bass_utils import run_bass_kernel_spmd

F32 = mybir.dt.float32
BF16 = mybir.dt.bfloat16
AF = mybir.ActivationFunctionType
ALU = mybir.AluOpType
AX = mybir.AxisListType

B, S, H = 2, 1024, 1024
T = B * S                 # 2048 tokens
I = 512                   # moe intermediate
E = 32                    # routed experts
TOP_K = 4
N_GROUP = 4
GRP = E // N_GROUP        # 8 experts per group
TOPK_GROUP = 2
SCALE = 2.5
SH_I = 1024               # shared intermediate (2 * I)
NCORES = 8
E_LOC = E // NCORES       # 4 experts per core
SH_LOC = SH_I // NCORES   # 128 shared-intermediate rows per core

P = 128
TC = 512                  # t-chunk (moving free dim)
NC_ = T // TC             # 4 t-chunks
NTT = TC // P             # 4 t-tiles per chunk
NH = H // P               # 8 h-tiles
NI = I // P               # 4 i-tiles per expert
# ReduceScatter piece boundaries (in h-tiles of 128 rows) per chunk:
# two even halves, so the first can start mid-way through the down phase.
PIECES = {c: [(0, 4), (4, 8)] for c in range(4)}


def _build(trace: bool = False):
    nc = bacc.Bacc("TRN2", target_bir_lowering=False, debug=False,
                   num_devices=NCORES)

    # ---- kernel I/O. All inputs are pre-packed on the host so every
    # DMA is a straight 2D copy with long contiguous runs per partition.
    tokf = nc.dram_tensor("tokf", [P, NC_ * NH * TC], F32,
                          kind="ExternalInput")
    tokb = nc.dram_tensor("tokb", [P, NC_ * NH * TC], BF16,
                          kind="ExternalInput")
    rwT = nc.dram_tensor("rwT", [P, NH * E], F32, kind="ExternalInput")
    rbias = nc.dram_tensor("rbias", [1, E], F32, kind="ExternalInput")
    ident = nc.dram_tensor("ident", [P, P], F32, kind="ExternalInput")
    selb = nc.dram_tensor("selb", [E, E_LOC * P], BF16,
                          kind="ExternalInput")
    gwT = nc.dram_tensor("gwT", [P, E_LOC * NH * I], BF16,
                         kind="ExternalInput")
    uwT = nc.dram_tensor("uwT", [P, E_LOC * NH * I], BF16,
                         kind="ExternalInput")
    dwT = nc.dram_tensor("dwT", [P, E_LOC * NI * H], BF16,
                         kind="ExternalInput")
    sgwT = nc.dram_tensor("sgwT", [P, NH * SH_LOC], BF16,
                          kind="ExternalInput")
    suwT = nc.dram_tensor("suwT", [P, NH * SH_LOC], BF16,
                          kind="ExternalInput")
    sdwT = nc.dram_tensor("sdwT", [SH_LOC, H], BF16, kind="ExternalInput")
    out_shard = nc.dram_tensor("out_shard", [P, T], BF16,
                               kind="ExternalOutput")

    rg = [list(range(NCORES))]

    with tile.TileContext(nc) as tc:
        with (
            tc.tile_pool(name="resident", bufs=1) as rp,
            tc.tile_pool(name="router", bufs=1) as rr,
            tc.tile_pool(name="work", bufs=2) as xp,
            tc.tile_pool(name="hid", bufs=1) as hp,
            tc.tile_pool(name="psum", bufs=2, space="PSUM") as ps,
            tc.tile_pool(name="dram", bufs=1, space="DRAM") as dp,
        ):
            # ---------- DMA priority order ----------
            # tiny consts first
            ident_sb = rp.tile([P, P], F32, tag="ident")
            nc.sync.dma_start(ident_sb[:], ident[:, :])
            rbias_sb = rp.tile([1, E], F32, tag="rbias")
            nc.sync.dma_start(rbias_sb[:], rbias[:, :])
            selb_sb = rp.tile([E, E_LOC, P], BF16, tag="selb")
            nc.sync.dma_start(selb_sb[:].rearrange("e l p -> e (l p)"),
                              selb[:, :])
            rw_sb = rp.tile([P, NH, E], F32, tag="rw")
            nc.sync.dma_start(rw_sb[:].rearrange("p a e -> p (a e)"),
                              rwT[:, :])

            # chunk-0 tokens. tokb (bf16, read all chunk long) is resident
            # per chunk; tokf (fp32, router-only) streams through a
            # 2-buffer ring.
            CW = NH * TC
            tokf_sb, tokb_sb = {}, {}

            def load_tokf(c):
                t_ = rp.tile([P, NH, TC], F32, tag="tokf", bufs=2,
                             name=f"tokf{c}")
                nc.sync.dma_start(t_[:].rearrange("p a t -> p (a t)"),
                                  tokf[:, c * CW:(c + 1) * CW])
                tokf_sb[c] = t_

            def load_tokb(c):
                t_ = rp.tile([P, NH, TC], BF16, tag="tokb", bufs=3,
                             name=f"tokb{c}")
                nc.sync.dma_start(t_[:].rearrange("p a t -> p (a t)"),
                                  tokb[:, c * CW:(c + 1) * CW])
                tokb_sb[c] = t_

            load_tokf(0)
            load_tokb(0)

            # shared-expert weights first (small, lets the PE start on the
            # shared matmuls while the bigger routed weights stream in)
            sgw_sb = rp.tile([P, NH, SH_LOC], BF16, tag="sgw")
            nc.sync.dma_start(sgw_sb[:].rearrange("p a s -> p (a s)"),
                              sgwT[:, :])
            suw_sb = rp.tile([P, NH, SH_LOC], BF16, tag="suw")
            nc.sync.dma_start(suw_sb[:].rearrange("p a s -> p (a s)"),
                              suwT[:, :])

            # gate/up weights, expert-major so expert 0 lands first
            EW = NH * I
            gw_sb, uw_sb = [], []
            for el in range(E_LOC):
                g_ = rp.tile([P, NH, I], BF16, tag=f"gw{el}")
                nc.sync.dma_start(g_[:].rearrange("p a i -> p (a i)"),
                                  gwT[:, el * EW:(el + 1) * EW])
                gw_sb.append(g_)
                u_ = rp.tile([P, NH, I], BF16, tag=f"uw{el}")
                nc.sync.dma_start(u_[:].rearrange("p a i -> p (a i)"),
                                  uwT[:, el * EW:(el + 1) * EW])
                uw_sb.append(u_)

            # chunk-1 tokens, then down weights (chunks 2-3 tokens are
            # prefetched from inside the chunk loop)
            load_tokf(1)
            load_tokb(1)
            dw_sb = rp.tile([P, E_LOC, NI, H], BF16, tag="dw")
            nc.sync.dma_start(dw_sb[:].rearrange("p l it h -> p (l it h)"),
                              dwT[:, :])
            sdw_sb = rp.tile([SH_LOC, H], BF16, tag="sdw")
            nc.sync.dma_start(sdw_sb[:], sdwT[:, :])

            # ---------- router bias broadcast [P, E] ----------
            ones = rp.tile([1, P], F32, tag="ones")
            nc.vector.memset(ones[:], 1.0)
            bias_ps = ps.tile([P, E], F32, tag="misc")
            nc.tensor.matmul(bias_ps[:], ones[:], rbias_sb[:],
                             start=True, stop=True)
            bias_b = rp.tile([P, E], F32, tag="bias_b")
            nc.scalar.copy(bias_b[:], bias_ps[:])

            # ---------- pipelined chunk loop ----------
            for c in range(NC_):
                tsl = slice(c * TC, (c + 1) * TC)
                tkf, tkb = tokf_sb[c], tokb_sb[c]

                # --- router scores: fp32 matmul + sigmoid per t-tile ---
                scores = rr.tile([P, NTT, E], F32, tag="scores")
                for tt in range(NTT):
                    sc_ps = ps.tile([P, E], F32, tag="misc")
                    for ht in range(NH):
                        nc.tensor.matmul(
                            sc_ps[:], tkf[:, ht, tt * P:(tt + 1) * P],
                            rw_sb[:, ht, :],
                            start=(ht == 0), stop=(ht == NH - 1))
                    nc.scalar.activation(scores[:, tt, :], sc_ps[:],
                                         AF.Sigmoid)

                # --- gate/up. silu is computed as g * sigmoid(g) * u so
                # the Act engine only ever uses the sigmoid table (a
                # Sigmoid<->Silu swap costs a 1.3us table load per switch).
                # The sigmoid (Act) + g*u (DVE) drain the PSUM banks right
                # behind each group without waiting on the router; the
                # routing weight is multiplied in once wb is ready. ---
                hid = {}

                def emit_swiglu(gp, up, h_):
                    # silu(g)*u = g*sigmoid(g)*u with each DVE op reading
                    # at most one PSUM operand
                    sg_t = xp.tile([P, TC], F32, tag="sg", name="sg_t")
                    nc.scalar.activation(sg_t[:], gp[:], AF.Sigmoid)
                    gu_t = xp.tile([P, TC], F32, tag="gu", name="gu_t",
                                   bufs=1)
                    nc.vector.tensor_tensor(gu_t[:], up[:], sg_t[:],
                                            op=ALU.mult)
                    nc.vector.tensor_tensor(h_[:], gu_t[:], gp[:],
                                            op=ALU.mult)

                def emit_gate_up(el):
                    for it in range(NI):
                        isl = slice(it * P, (it + 1) * P)
                        gp = ps.tile([P, TC], F32, tag="g_ps", name="gp")
                        up = ps.tile([P, TC], F32, tag="u_ps", name="up")
                        for ht in range(NH):
                            nc.tensor.matmul(gp[:], gw_sb[el][:, ht, isl],
                                             tkb[:, ht, :],
                                             start=(ht == 0),
                                             stop=(ht == NH - 1))
                        for ht in range(NH):
                            nc.tensor.matmul(up[:], uw_sb[el][:, ht, isl],
                                             tkb[:, ht, :],
                                             start=(ht == 0),
                                             stop=(ht == NH - 1))
                        h_ = hp.tile([P, TC], BF16, tag=f"hid{el}_{it}",
                                     name=f"hid{el}_{it}")
                        emit_swiglu(gp, up, h_)
                        hid[(el, it)] = h_

                # shared expert first: its weights land earliest and its
                # hidden state has no routing-weight dependency at all
                sg_ps = ps.tile([P, TC], F32, tag="g_ps")
                su_ps = ps.tile([P, TC], F32, tag="u_ps")
                for ht in range(NH):
                    nc.tensor.matmul(sg_ps[:], sgw_sb[:, ht, :],
                                     tkb[:, ht, :],
                                     start=(ht == 0), stop=(ht == NH - 1))
                for ht in range(NH):
                    nc.tensor.matmul(su_ps[:], suw_sb[:, ht, :],
                                     tkb[:, ht, :],
                                     start=(ht == 0), stop=(ht == NH - 1))
                sh_hid = hp.tile([P, TC], BF16, tag="sh_hid")
                emit_swiglu(sg_ps, su_ps, sh_hid)

                emit_gate_up(0)

                # --- batched top-k router chain (DVE) ---
                def r3(t_):
                    return t_[:]
                def r4(t_):
                    return t_[:].rearrange("p a (g e) -> p a g e", e=GRP)

                sfc = rr.tile([P, NTT, E], F32, tag="sfc")
                nc.vector.tensor_tensor(
                    sfc[:], scores[:],
                    bias_b[:].unsqueeze(1).broadcast_to([P, NTT, E]),
                    op=ALU.add)
                m1 = rr.tile([P, NTT, N_GROUP], F32, tag="m1")
                nc.vector.tensor_reduce(m1[:], r4(sfc), axis=AX.X,
                                        op=ALU.max)
                eq = rr.tile([P, NTT, E], F32, tag="eq")
                nc.vector.tensor_tensor(
                    r4(eq), r4(sfc),
                    m1[:].unsqueeze(3).broadcast_to([P, NTT, N_GROUP, GRP]),
                    op=ALU.is_equal)
                tmp = rr.tile([P, NTT, E], F32, tag="tmp")
                nc.vector.tensor_scalar_mul(tmp[:], eq[:], 1e30)
                wo = rr.tile([P, NTT, E], F32, tag="wo")
                nc.vector.tensor_tensor(wo[:], sfc[:], tmp[:],
                                        op=ALU.subtract)
                m2 = rr.tile([P, NTT, N_GROUP], F32, tag="m2")
                nc.vector.tensor_reduce(m2[:], r4(wo), axis=AX.X, op=ALU.max)
                gs = rr.tile([P, NTT, N_GROUP], F32, tag="gs")
                nc.vector.tensor_tensor(gs[:], m1[:], m2[:], op=ALU.add)
                gm1 = rr.tile([P, NTT], F32, tag="gm1")
                nc.vector.tensor_reduce(gm1[:], gs[:], axis=AX.X, op=ALU.max)
                eqg = rr.tile([P, NTT, N_GROUP], F32, tag="eqg")
                nc.vector.tensor_tensor(
                    eqg[:], gs[:],
                    gm1[:].unsqueeze(2).broadcast_to([P, NTT, N_GROUP]),
                    op=ALU.is_equal)
                tmpg = rr.tile([P, NTT, N_GROUP], F32, tag="tmpg")
                nc.vector.tensor_scalar_mul(tmpg[:], eqg[:], 1e30)
                gs2 = rr.tile([P, NTT, N_GROUP], F32, tag="gs2")
                nc.vector.tensor_tensor(gs2[:], gs[:], tmpg[:],
                                        op=ALU.subtract)
                gm2 = rr.tile([P, NTT], F32, tag="gm2")
                nc.vector.tensor_reduce(gm2[:], gs2[:], axis=AX.X,
                                        op=ALU.max)
                gmask = rr.tile([P, NTT, N_GROUP], F32, tag="gmask")
                nc.vector.tensor_tensor(
                    gmask[:], gs[:],
                    gm2[:].unsqueeze(2).broadcast_to([P, NTT, N_GROUP]),
                    op=ALU.is_ge)
                masked = rr.tile([P, NTT, E], F32, tag="masked")
                nc.vector.tensor_tensor(
                    r4(masked), r4(sfc),
                    gmask[:].unsqueeze(3).broadcast_to(
                        [P, NTT, N_GROUP, GRP]),
                    op=ALU.mult)
                mx = rr.tile([P, NTT * 8], F32, tag="mx")
                for tt in range(NTT):
                    nc.vector.max(mx[:, tt * 8:(tt + 1) * 8],
                                  masked[:, tt, :])
                m4b = (mx[:].rearrange("p (a k) -> p a k", k=8)[:, :, 3:4]
                       .broadcast_to([P, NTT, E]))
                sel = rr.tile([P, NTT, E], F32, tag="sel")
                nc.vector.tensor_tensor(sel[:], masked[:], m4b,
                                        op=ALU.is_ge)
                wun = rr.tile([P, NTT, E], F32, tag="wun")
                nc.vector.tensor_tensor(wun[:], scores[:], sel[:],
                                        op=ALU.mult)
                den = rr.tile([P, NTT], F32, tag="den")
                nc.vector.tensor_reduce(den[:], wun[:], axis=AX.X,
                                        op=ALU.add)
                nc.vector.tensor_scalar_add(den[:], den[:], 1e-20)
                rec = rr.tile([P, NTT], F32, tag="rec")
                nc.vector.reciprocal(rec[:], den[:])
                nc.vector.tensor_scalar_mul(rec[:], rec[:], SCALE)
                wfin = rr.tile([P, NTT, E], F32, tag="wfin")
                nc.vector.tensor_tensor(
                    wfin[:], wun[:],
                    rec[:].unsqueeze(2).broadcast_to([P, NTT, E]),
                    op=ALU.mult)

                # --- wt = wfin^T (PE transpose) then per-expert broadcast ---
                wt_sb = rr.tile([E, NTT, P], BF16, tag="wt")
                for tt in range(NTT):
                    wt_ps = ps.tile([E, P], F32, tag="misc")
                    nc.tensor.transpose(wt_ps[:], wfin[:, tt, :],
                                        ident_sb[:])
                    nc.scalar.copy(wt_sb[:, tt, :], wt_ps[:])
                wb_sb = {}
                for el in range(E_LOC):
                    wb_ps = ps.tile([P, TC], F32, tag="misc")
                    nc.tensor.matmul(
                        wb_ps[:], selb_sb[:, el, :],
                        wt_sb[:].rearrange("e a t -> e (a t)"),
                        start=True, stop=True)
                    w_ = xp.tile([P, TC], F32, tag="wb", bufs=4)
                    nc.scalar.copy(w_[:], wb_ps[:])
                    wb_sb[el] = w_

                # --- experts 1-3 gate/up ---
                for el in range(1, E_LOC):
                    emit_gate_up(el)

                # --- scale hid by routing weights (in place, DVE) ---
                for el in range(E_LOC):
                    for it in range(NI):
                        h_ = hid[(el, it)]
                        nc.vector.tensor_tensor(h_[:], h_[:],
                                                wb_sb[el][:], op=ALU.mult)

                # prefetch tokens two chunks ahead. Emitted here (not at
                # the chunk top) so the DMA issues land mid-chunk, away
                # from the previous chunk's in-flight ReduceScatter.
                if c + 2 < NC_:
                    load_tokf(c + 2)
                    load_tokb(c + 2)

                # --- down matmuls; bf16 partials to DRAM; two RS pieces
                # per chunk. The last chunk uses an uneven 6+2 split so
                # the only fully exposed collective is a small one. ---
                pieces = PIECES[c]
                cc_in = [dp.tile([(b - a) * P, TC], BF16,
                                 tag=f"cc_in{c}_{k}", name=f"cc_in{c}_{k}")
                         for k, (a, b) in enumerate(pieces)]
                for ht in range(NH):
                    hsl = slice(ht * P, (ht + 1) * P)
                    d_ps = ps.tile([P, TC], F32, tag="d_ps")
                    k = 0
                    for el in range(E_LOC):
                        for it in range(NI):
                            nc.tensor.matmul(d_ps[:],
                                             dw_sb[:, el, it, hsl],
                                             hid[(el, it)][:],
                                             start=(k == 0), stop=False)
                            k += 1
                    nc.tensor.matmul(d_ps[:], sdw_sb[:, hsl], sh_hid[:],
                                     start=False, stop=True)
                    o_sb = xp.tile([P, TC], BF16, tag="o_sb")
                    nc.vector.tensor_copy(o_sb[:], d_ps[:])
                    piece = next(k for k, (a, b) in enumerate(pieces)
                                 if a <= ht < b)
                    a, b = pieces[piece]
                    nc.sync.dma_start(cc_in[piece][(ht - a) * P:
                                                   (ht - a + 1) * P, :],
                                      o_sb[:])
                    if ht == b - 1:
                        rows = (b - a) * P // NCORES
                        cc_out = dp.tile([rows, TC], BF16,
                                         tag=f"cc_out{c}_{piece}",
                                         name=f"cc_out{c}_{piece}")
                        nc.gpsimd.collective_compute(
                            "ReduceScatter", ALU.add, replica_groups=rg,
                            ins=[cc_in[piece].opt()], outs=[cc_out.opt()],
                        )
                        nc.gpsimd.dma_start(
                            out_shard[a * P // NCORES:
                                      a * P // NCORES + rows, tsl],
                            cc_out[:])

    nc.compile()
    return nc


def _pack_rows(a):
    """[X*128, Y] row-major -> [128, X*Y] with per-partition layout (X, Y)."""
    X = a.shape[0] // P
    return np.ascontiguousarray(
        a.reshape(X, P, -1).transpose(1, 0, 2).reshape(P, -1))


def _prep_inputs(hidden_states, router_weight, router_bias, gate_w, up_w,
                 down_w, shared_gate_w, shared_up_w, shared_down_w):
    bf = ml_dtypes.bfloat16
    tokens = np.ascontiguousarray(
        np.asarray(hidden_states, dtype=np.float32).reshape(T, H))
    tokf = np.ascontiguousarray(tokens.T)                       # [H, T] f32
    # pack tokens chunk-major: [128, (chunk, h-tile, t)]
    tokf_p = np.ascontiguousarray(
        tokf.reshape(NH, P, NC_, TC).transpose(1, 2, 0, 3).reshape(P, -1))
    tokb_p = tokf_p.astype(bf)
    rw_p = _pack_rows(np.ascontiguousarray(
        np.asarray(router_weight, dtype=np.float32).T))         # [H, E]
    rbias = np.asarray(router_bias, dtype=np.float32).reshape(1, E)
    ident = np.eye(P, dtype=np.float32)
    gwT = np.ascontiguousarray(
        np.asarray(gate_w, dtype=np.float32).transpose(0, 2, 1)).astype(bf)
    uwT = np.ascontiguousarray(
        np.asarray(up_w, dtype=np.float32).transpose(0, 2, 1)).astype(bf)
    dwT = np.ascontiguousarray(
        np.asarray(down_w, dtype=np.float32).transpose(0, 2, 1)).astype(bf)
    sgwT = np.ascontiguousarray(
        np.asarray(shared_gate_w, dtype=np.float32).T)          # [H, SH_I]
    suwT = np.ascontiguousarray(
        np.asarray(shared_up_w, dtype=np.float32).T)
    sdwT = np.ascontiguousarray(
        np.asarray(shared_down_w, dtype=np.float32).T)          # [SH_I, H]

    in_maps = []
    for c in range(NCORES):
        esl = slice(c * E_LOC, (c + 1) * E_LOC)
        ssl = slice(c * SH_LOC, (c + 1) * SH_LOC)
        sel = np.zeros((E_LOC, E, P), dtype=np.float32)
        for el in range(E_LOC):
            sel[el, c * E_LOC + el, :] = 1.0
        # gw/uw: [128, (el, h-tile, i)]
        gw_loc = gwT[esl]            # [E_LOC, H, I]
        gw_p = np.ascontiguousarray(
            gw_loc.reshape(E_LOC, NH, P, I).transpose(2, 0, 1, 3)
            .reshape(P, -1))
        uw_loc = uwT[esl]
        uw_p = np.ascontiguousarray(
            uw_loc.reshape(E_LOC, NH, P, I).transpose(2, 0, 1, 3)
            .reshape(P, -1))
        # dw: [128, (el, i-tile, h)]
        dw_loc = dwT[esl]            # [E_LOC, I, H]
        dw_p = np.ascontiguousarray(
            dw_loc.reshape(E_LOC, NI, P, H).transpose(2, 0, 1, 3)
            .reshape(P, -1))
        in_maps.append({
            "tokf": tokf_p,
            "tokb": tokb_p,
            "rwT": rw_p,
            "rbias": rbias,
            "ident": ident,
            "selb": np.ascontiguousarray(
                sel.transpose(1, 0, 2).reshape(E, -1)).astype(bf),
            "gwT": gw_p,
            "uwT": uw_p,
            "dwT": dw_p,
            "sgwT": _pack_rows(np.ascontiguousarray(sgwT[:, ssl])
                               .astype(bf)),
            "suwT": _pack_rows(np.ascontiguousarray(suwT[:, ssl])
                               .astype(bf)),
            "sdwT": np.ascontiguousarray(sdwT[ssl, :]).astype(bf),
        })
    return in_maps


def run_on_device(inputs: dict, trace: bool = False, tmpdir: str | None = None):
    in_maps = _prep_inputs(**inputs)
    nc = _build(trace=trace)
    res = run_bass_kernel_spmd(nc, in_maps, list(range(NCORES)), trace=trace,
                               tmpdir=tmpdir)
    # Reassemble: for RS piece (a, b) of chunk ck, core c's shard rows
    # [16a, 16a + 16(b-a)) hold global h rows [128a + 16(b-a)c, ...).
    outT = np.empty((H, T), dtype=np.float32)
    for c in range(NCORES):
        sh = np.asarray(res.results[c]["out_shard"], dtype=np.float32)
        for ck in range(NC_):
            csl = slice(ck * TC, (ck + 1) * TC)
            for (a, b) in PIECES[ck]:
                w = (b - a) * P // NCORES
                outT[a * P + c * w:a * P + (c + 1) * w, csl] = \
                    sh[a * P // NCORES:a * P // NCORES + w, csl]
    out = np.ascontiguousarray(outT.T).reshape(B, S, H).astype(np.float32)
    return out, res


def kernel(**inputs) -> np.ndarray:
    out, _ = run_on_device(inputs, trace=False)
    return out


# revision 38
# speedup vs baseline: 1.0562x; 1.0493x over previous
"""Kimi-K2.5 tensorized MoE kernel for 8 TRN2 NeuronCores.

Sharding: expert-parallel. Core c owns routed experts [4c, 4c+4) and rows
[128c, 128(c+1)) of the shared-expert intermediate. The router runs
replicated on every core in fp32 (selection must match the reference
exactly), pipelined per 512-token chunk with the expert matmuls; the
top-k logic is batched across the chunk's four 128-token tiles with
multi-dim access patterns so it stays off the PE critical path.
Expert/shared matmuls run in bf16 with fp32 PSUM accumulation.

Per-core partial outputs are reduced on-device with bf16 ReduceScatters,
two per 512-token chunk (h rows [0,512) and [512,1024) separately so the
first collective can start mid-chunk). With the h-split, core c ends up
holding rows [64c, 64c+64) of the first h-half and rows [512+64c,
512+64c+64) of the second for every token column; the host reassembles
accordingly and transposes back to [B, S, H] fp32.
"""

import os
import sys

sys.path.insert(0, "/opt/trn_rl_repo")

import numpy as np
import ml_dtypes

from concourse import bass, bacc, mybir, tile
from concourse.bass_utils import run_bass_kernel_spmd

F32 = mybir.dt.float32
BF16 = mybir.dt.bfloat16
AF = mybir.ActivationFunctionType
ALU = mybir.AluOpType
AX = mybir.AxisListType

B, S, H = 2, 1024, 1024
T = B * S                 # 2048 tokens
I = 512                   # moe intermediate
E = 32                    # routed experts
TOP_K = 4
N_GROUP = 4
GRP = E // N_GROUP        # 8 experts per group
TOPK_GROUP = 2
SCALE = 2.5
SH_I = 1024               # shared intermediate (2 * I)
NCORES = 8
E_LOC = E // NCORES       # 4 experts per core
SH_LOC = SH_I // NCORES   # 128 shared-intermediate rows per core

P = 128
TC = 512                  # t-chunk (moving free dim)
NC_ = T // TC             # 4 t-chunks
NTT = TC // P             # 4 t-tiles per chunk
NH = H // P               # 8 h-tiles
NI = I // P               # 4 i-tiles per expert
# ReduceScatter piece boundaries (in h-tiles of 128 rows) per chunk:
# two even halves, so the first can start mid-way through the down phase.
PIECES = {c: [(0, 4), (4, 8)] for c in range(4)}


def _build(trace: bool = False):
    nc = bacc.Bacc("TRN2", target_bir_lowering=False, debug=False,
                   num_devices=NCORES)

    # ---- kernel I/O. All inputs are pre-packed on the host so every
    # DMA is a straight 2D copy with long contiguous runs per partition.
    tokf = nc.dram_tensor("tokf", [P, NC_ * NH * TC], F32,
                          kind="ExternalInput")
    tokb = nc.dram_tensor("tokb", [P, NC_ * NH * TC], BF16,
                          kind="ExternalInput")
    rwT = nc.dram_tensor("rwT", [P, NH * E], F32, kind="ExternalInput")
    rbias = nc.dram_tensor("rbias", [1, E], F32, kind="ExternalInput")
    ident = nc.dram_tensor("ident", [P, P], F32, kind="ExternalInput")
    selb = nc.dram_tensor("selb", [E, E_LOC * P], BF16,
                          kind="ExternalInput")
    gwT = nc.dram_tensor("gwT", [P, E_LOC * NH * I], BF16,
                         kind="ExternalInput")
    uwT = nc.dram_tensor("uwT", [P, E_LOC * NH * I], BF16,
                         kind="ExternalInput")
    dwT = nc.dram_tensor("dwT", [P, E_LOC * NI * H], BF16,
                         kind="ExternalInput")
    sgwT = nc.dram_tensor("sgwT", [P, NH * SH_LOC], BF16,
                          kind="ExternalInput")
    suwT = nc.dram_tensor("suwT", [P, NH * SH_LOC], BF16,
                          kind="ExternalInput")
    sdwT = nc.dram_tensor("sdwT", [SH_LOC, H], BF16, kind="ExternalInput")
    out_shard = nc.dram_tensor("out_shard", [P, T], BF16,
                               kind="ExternalOutput")

    rg = [list(range(NCORES))]

    with tile.TileContext(nc) as tc:
        with (
            tc.tile_pool(name="resident", bufs=1) as rp,
            tc.tile_pool(name="router", bufs=1) as rr,
            tc.tile_pool(name="work", bufs=2) as xp,
            tc.tile_pool(name="hid", bufs=1) as hp,
            tc.tile_pool(name="psum", bufs=2, space="PSUM") as ps,
            tc.tile_pool(name="dram", bufs=1, space="DRAM") as dp,
        ):
            # ---------- DMA priority order ----------
            # tiny consts first
            ident_sb = rp.tile([P, P], F32, tag="ident")
            nc.sync.dma_start(ident_sb[:], ident[:, :])
            rbias_sb = rp.tile([1, E], F32, tag="rbias")
            nc.sync.dma_start(rbias_sb[:], rbias[:, :])
            selb_sb = rp.tile([E, E_LOC, P], BF16, tag="selb")
            nc.sync.dma_start(selb_sb[:].rearrange("e l p -> e (l p)"),
                              selb[:, :])
            rw_sb = rp.tile([P, NH, E], F32, tag="rw")
            nc.sync.dma_start(rw_sb[:].rearrange("p a e -> p (a e)"),
                              rwT[:, :])

            # chunk-0 tokens. tokb (bf16, read all chunk long) is resident
            # per chunk; tokf (fp32, router-only) streams through a
            # 2-buffer ring.
            CW = NH * TC
            tokf_sb, tokb_sb = {}, {}

            def load_tokf(c):
                t_ = rp.tile([P, NH, TC], F32, tag="tokf", bufs=2,
                             name=f"tokf{c}")
                nc.sync.dma_start(t_[:].rearrange("p a t -> p (a t)"),
                                  tokf[:, c * CW:(c + 1) * CW])
                tokf_sb[c] = t_

            def load_tokb(c):
                t_ = rp.tile([P, NH, TC], BF16, tag="tokb", bufs=3,
                             name=f"tokb{c}")
                nc.sync.dma_start(t_[:].rearrange("p a t -> p (a t)"),
                                  tokb[:, c * CW:(c + 1) * CW])
                tokb_sb[c] = t_

            load_tokf(0)
            load_tokb(0)

            # shared-expert weights first (small, lets the PE start on the
            # shared matmuls while the bigger routed weights stream in)
            sgw_sb = rp.tile([P, NH, SH_LOC], BF16, tag="sgw")
            nc.sync.dma_start(sgw_sb[:].rearrange("p a s -> p (a s)"),
                              sgwT[:, :])
            suw_sb = rp.tile([P, NH, SH_LOC], BF16, tag="suw")
            nc.sync.dma_start(suw_sb[:].rearrange("p a s -> p (a s)"),
                              suwT[:, :])

            # gate/up weights, expert-major so expert 0 lands first
            EW = NH * I
            gw_sb, uw_sb = [], []
            for el in range(E_LOC):
                g_ = rp.tile([P, NH, I], BF16, tag=f"gw{el}")
                nc.sync.dma_start(g_[:].rearrange("p a i -> p (a i)"),
                                  gwT[:, el * EW:(el + 1) * EW])
                gw_sb.append(g_)
                u_ = rp.tile([P, NH, I], BF16, tag=f"uw{el}")
                nc.sync.dma_start(u_[:].rearrange("p a i -> p (a i)"),
                                  uwT[:, el * EW:(el + 1) * EW])
                uw_sb.append(u_)

            # chunk-1 tokens, then down weights (chunks 2-3 tokens are
            # prefetched from inside the chunk loop)
            load_tokf(1)
            load_tokb(1)
            dw_sb = rp.tile([P, E_LOC, NI, H], BF16, tag="dw")
            nc.sync.dma_start(dw_sb[:].rearrange("p l it h -> p (l it h)"),
                              dwT[:, :])
            sdw_sb = rp.tile([SH_LOC, H], BF16, tag="sdw")
            nc.sync.dma_start(sdw_sb[:], sdwT[:, :])

            # ---------- router bias broadcast [P, E] ----------
            ones = rp.tile([1, P], F32, tag="ones")
            nc.vector.memset(ones[:], 1.0)
            bias_ps = ps.tile([P, E], F32, tag="misc")
            nc.tensor.matmul(bias_ps[:], ones[:], rbias_sb[:],
                             start=True, stop=True)
            bias_b = rp.tile([P, E], F32, tag="bias_b")
            nc.scalar.copy(bias_b[:], bias_ps[:])

            # ---------- pipelined chunk loop ----------
            for c in range(NC_):
                tsl = slice(c * TC, (c + 1) * TC)
                tkf, tkb = tokf_sb[c], tokb_sb[c]

                # --- router scores: fp32 matmul + sigmoid per t-tile ---
                scores = rr.tile([P, NTT, E], F32, tag="scores")
                for tt in range(NTT):
                    sc_ps = ps.tile([P, E], F32, tag="misc")
                    for ht in range(NH):
                        nc.tensor.matmul(
                            sc_ps[:], tkf[:, ht, tt * P:(tt + 1) * P],
                            rw_sb[:, ht, :],
                            start=(ht == 0), stop=(ht == NH - 1))
                    nc.scalar.activation(scores[:, tt, :], sc_ps[:],
                                         AF.Sigmoid)

                # --- gate/up. silu is computed as g * sigmoid(g) * u so
                # the Act engine only ever uses the sigmoid table (a
                # Sigmoid<->Silu swap costs a 1.3us table load per switch).
                # The sigmoid (Act) + g*u (DVE) drain the PSUM banks right
                # behind each group without waiting on the router; the
                # routing weight is multiplied in once wb is ready. ---
                hid = {}

                def emit_swiglu(gp, up, h_):
                    # silu(g)*u = g*sigmoid(g)*u with each DVE op reading
                    # at most one PSUM operand
                    sg_t = xp.tile([P, TC], F32, tag="sg", name="sg_t")
                    nc.scalar.activation(sg_t[:], gp[:], AF.Sigmoid)
                    gu_t = xp.tile([P, TC], F32, tag="gu", name="gu_t",
                                   bufs=1)
                    nc.vector.tensor_tensor(gu_t[:], up[:], sg_t[:],
                                            op=ALU.mult)
                    nc.vector.tensor_tensor(h_[:], gu_t[:], gp[:],
                                            op=ALU.mult)

                def emit_gate_up(el):
                    for it in range(NI):
                        isl = slice(it * P, (it + 1) * P)
                        gp = ps.tile([P, TC], F32, tag="g_ps", name="gp")
                        up = ps.tile([P, TC], F32, tag="u_ps", name="up")
                        for ht in range(NH):
                            nc.tensor.matmul(gp[:], gw_sb[el][:, ht, isl],
                                             tkb[:, ht, :],
                                             start=(ht == 0),
                                             stop=(ht == NH - 1))
                        for ht in range(NH):
                            nc.tensor.matmul(up[:], uw_sb[el][:, ht, isl],
                                             tkb[:, ht, :],
                                             start=(ht == 0),
                                             stop=(ht == NH - 1))
                        h_ = hp.tile([P, TC], BF16, tag=f"hid{el}_{it}",
                                     name=f"hid{el}_{it}")
                        emit_swiglu(gp, up, h_)
                        hid[(el, it)] = h_

                # shared expert first: its weights land earliest and its
                # hidden state has no routing-weight dependency at all
                sg_ps = ps.tile([P, TC], F32, tag="g_ps")
                su_ps = ps.tile([P, TC], F32, tag="u_ps")
                for ht in range(NH):
                    nc.tensor.matmul(sg_ps[:], sgw_sb[:, ht, :],
                                     tkb[:, ht, :],
                                     start=(ht == 0), stop=(ht == NH - 1))
                for ht in range(NH):
                    nc.tensor.matmul(su_ps[:], suw_sb[:, ht, :],
                                     tkb[:, ht, :],
                                     start=(ht == 0), stop=(ht == NH - 1))
                sh_hid = hp.tile([P, TC], BF16, tag="sh_hid")
                emit_swiglu(sg_ps, su_ps, sh_hid)

                emit_gate_up(0)

                # --- batched top-k router chain (DVE) ---
                def r3(t_):
                    return t_[:]
                def r4(t_):
                    return t_[:].rearrange("p a (g e) -> p a g e", e=GRP)

                sfc = rr.tile([P, NTT, E], F32, tag="sfc")
                nc.vector.tensor_tensor(
                    sfc[:], scores[:],
                    bias_b[:].unsqueeze(1).broadcast_to([P, NTT, E]),
                    op=ALU.add)
                m1 = rr.tile([P, NTT, N_GROUP], F32, tag="m1")
                nc.vector.tensor_reduce(m1[:], r4(sfc), axis=AX.X,
                                        op=ALU.max)
                eq = rr.tile([P, NTT, E], F32, tag="eq")
                nc.vector.tensor_tensor(
                    r4(eq), r4(sfc),
                    m1[:].unsqueeze(3).broadcast_to([P, NTT, N_GROUP, GRP]),
                    op=ALU.is_equal)
                tmp = rr.tile([P, NTT, E], F32, tag="tmp")
                nc.vector.tensor_scalar_mul(tmp[:], eq[:], 1e30)
                wo = rr.tile([P, NTT, E], F32, tag="wo")
                nc.vector.tensor_tensor(wo[:], sfc[:], tmp[:],
                                        op=ALU.subtract)
                m2 = rr.tile([P, NTT, N_GROUP], F32, tag="m2")
                nc.vector.tensor_reduce(m2[:], r4(wo), axis=AX.X, op=ALU.max)
                gs = rr.tile([P, NTT, N_GROUP], F32, tag="gs")
                nc.vector.tensor_tensor(gs[:], m1[:], m2[:], op=ALU.add)
                gm1 = rr.tile([P, NTT], F32, tag="gm1")
                nc.vector.tensor_reduce(gm1[:], gs[:], axis=AX.X, op=ALU.max)
                eqg = rr.tile([P, NTT, N_GROUP], F32, tag="eqg")
                nc.vector.tensor_tensor(
                    eqg[:], gs[:],
                    gm1[:].unsqueeze(2).broadcast_to([P, NTT, N_GROUP]),
                    op=ALU.is_equal)
                tmpg = rr.tile([P, NTT, N_GROUP], F32, tag="tmpg")
                nc.vector.tensor_scalar_mul(tmpg[:], eqg[:], 1e30)
                gs2 = rr.tile([P, NTT, N_GROUP], F32, tag="gs2")
                nc.vector.tensor_tensor(gs2[:], gs[:], tmpg[:],
                                        op=ALU.subtract)
                gm2 = rr.tile([P, NTT], F32, tag="gm2")
                nc.vector.tensor_reduce(gm2[:], gs2[:], axis=AX.X,
                                        op=ALU.max)
                gmask = rr.tile([P, NTT, N_GROUP], F32, tag="gmask")
                nc.vector.tensor_tensor(
                    gmask[:], gs[:],
                    gm2[:].unsqueeze(2).broadcast_to([P, NTT, N_GROUP]),
                    op=ALU.is_ge)
                masked = rr.tile([P, NTT, E], F32, tag="masked")
                nc.vector.tensor_tensor(
                    r4(masked), r4(sfc),
                    gmask[:].unsqueeze(3).broadcast_to(
                        [P, NTT, N_GROUP, GRP]),
                    op=ALU.mult)
                mx = rr.tile([P, NTT * 8], F32, tag="mx")
                for tt in range(NTT):
                    nc.vector.max(mx[:, tt * 8:(tt + 1) * 8],
                                  masked[:, tt, :])
                m4b = (mx[:].rearrange("p (a k) -> p a k", k=8)[:, :, 3:4]
                       .broadcast_to([P, NTT, E]))
                sel = rr.tile([P, NTT, E], F32, tag="sel")
                nc.vector.tensor_tensor(sel[:], masked[:], m4b,
                                        op=ALU.is_ge)
                wun = rr.tile([P, NTT, E], F32, tag="wun")
                nc.vector.tensor_tensor(wun[:], scores[:], sel[:],
                                        op=ALU.mult)
                den = rr.tile([P, NTT], F32, tag="den")
                nc.vector.tensor_reduce(den[:], wun[:], axis=AX.X,
                                        op=ALU.add)
                nc.vector.tensor_scalar_add(den[:], den[:], 1e-20)
                rec = rr.tile([P, NTT], F32, tag="rec")
                nc.vector.reciprocal(rec[:], den[:])
                nc.vector.tensor_scalar_mul(rec[:], rec[:], SCALE)
                wfin = rr.tile([P, NTT, E], F32, tag="wfin")
                nc.vector.tensor_tensor(
                    wfin[:], wun[:],
                    rec[:].unsqueeze(2).broadcast_to([P, NTT, E]),
                    op=ALU.mult)

                # --- wt = wfin^T (PE transpose) then per-expert broadcast ---
                wt_sb = rr.tile([E, NTT, P], BF16, tag="wt")
                for tt in range(NTT):
                    wt_ps = ps.tile([E, P], F32, tag="misc")
                    nc.tensor.transpose(wt_ps[:], wfin[:, tt, :],
                                        ident_sb[:])
                    nc.scalar.copy(wt_sb[:, tt, :], wt_ps[:])
                wb_sb = {}
                for el in range(E_LOC):
                    wb_ps = ps.tile([P, TC], F32, tag="misc")
                    nc.tensor.matmul(
                        wb_ps[:], selb_sb[:, el, :],
                        wt_sb[:].rearrange("e a t -> e (a t)"),
                        start=True, stop=True)
                    w_ = xp.tile([P, TC], F32, tag="wb", bufs=4)
                    nc.scalar.copy(w_[:], wb_ps[:])
                    wb_sb[el] = w_

                # --- experts 1-3 gate/up ---
                for el in range(1, E_LOC):
                    emit_gate_up(el)

                # --- scale hid by routing weights (in place, DVE) ---
                for el in range(E_LOC):
                    for it in range(NI):
                        h_ = hid[(el, it)]
                        nc.vector.tensor_tensor(h_[:], h_[:],
                                                wb_sb[el][:], op=ALU.mult)

                # prefetch tokens two chunks ahead. Emitted here (not at
                # the chunk top) so the DMA issues land mid-chunk, away
                # from the previous chunk's in-flight ReduceScatter.
                if c + 2 < NC_:
                    load_tokf(c + 2)
                    load_tokb(c + 2)

                # --- down matmuls; bf16 partials to DRAM; two RS pieces
                # per chunk. The last chunk uses an uneven 6+2 split so
                # the only fully exposed collective is a small one. ---
                pieces = PIECES[c]
                cc_in = [dp.tile([(b - a) * P, TC], BF16,
                                 tag=f"cc_in{c}_{k}", name=f"cc_in{c}_{k}")
                         for k, (a, b) in enumerate(pieces)]
                for ht in range(NH):
                    hsl = slice(ht * P, (ht + 1) * P)
                    d_ps = ps.tile([P, TC], F32, tag="d_ps")
                    k = 0
                    for el in range(E_LOC):
                        for it in range(NI):
                            nc.tensor.matmul(d_ps[:],
                                             dw_sb[:, el, it, hsl],
                                             hid[(el, it)][:],
                                             start=(k == 0), stop=False)
                            k += 1
                    nc.tensor.matmul(d_ps[:], sdw_sb[:, hsl], sh_hid[:],
                                     start=False, stop=True)
                    o_sb = xp.tile([P, TC], BF16, tag="o_sb")
                    nc.vector.tensor_copy(o_sb[:], d_ps[:])
                    piece = next(k for k, (a, b) in enumerate(pieces)
                                 if a <= ht < b)
                    a, b = pieces[piece]
                    nc.sync.dma_start(cc_in[piece][(ht - a) * P:
                                                   (ht - a + 1) * P, :],
                                      o_sb[:])
                    if ht == b - 1:
                        rows = (b - a) * P // NCORES
                        cc_out = dp.tile([rows, TC], BF16,
                                         tag=f"cc_out{c}_{piece}",
                                         name=f"cc_out{c}_{piece}")
                        nc.gpsimd.collective_compute(
                            "ReduceScatter", ALU.add, replica_groups=rg,
                            ins=[cc_in[piece].opt()], outs=[cc_out.opt()],
                        )
                        nc.gpsimd.dma_start(
                            out_shard[a * P // NCORES:
                                      a * P // NCORES + rows, tsl],
                            cc_out[:])

    nc.compile()
    return nc


def _pack_rows(a):
    """[X*128, Y] row-major -> [128, X*Y] with per-partition layout (X, Y)."""
    X = a.shape[0] // P
    return np.ascontiguousarray(
        a.reshape(X, P, -1).transpose(1, 0, 2).reshape(P, -1))


def _prep_inputs(hidden_states, router_weight, router_bias, gate_w, up_w,
                 down_w, shared_gate_w, shared_up_w, shared_down_w):
    bf = ml_dtypes.bfloat16
    tokens = np.ascontiguousarray(
        np.asarray(hidden_states, dtype=np.float32).reshape(T, H))
    tokf = np.ascontiguousarray(tokens.T)                       # [H, T] f32
    # pack tokens chunk-major: [128, (chunk, h-tile, t)]
    tokf_p = np.ascontiguousarray(
        tokf.reshape(NH, P, NC_, TC).transpose(1, 2, 0, 3).reshape(P, -1))
    tokb_p = tokf_p.astype(bf)
    rw_p = _pack_rows(np.ascontiguousarray(
        np.asarray(router_weight, dtype=np.float32).T))         # [H, E]
    rbias = np.asarray(router_bias, dtype=np.float32).reshape(1, E)
    ident = np.eye(P, dtype=np.float32)
    gwT = np.ascontiguousarray(
        np.asarray(gate_w, dtype=np.float32).transpose(0, 2, 1)).astype(bf)
    uwT = np.ascontiguousarray(
        np.asarray(up_w, dtype=np.float32).transpose(0, 2, 1)).astype(bf)
    dwT = np.ascontiguousarray(
        np.asarray(down_w, dtype=np.float32).transpose(0, 2, 1)).astype(bf)
    sgwT = np.ascontiguousarray(
        np.asarray(shared_gate_w, dtype=np.float32).T)          # [H, SH_I]
    suwT = np.ascontiguousarray(
        np.asarray(shared_up_w, dtype=np.float32).T)
    sdwT = np.ascontiguousarray(
        np.asarray(shared_down_w, dtype=np.float32).T)          # [SH_I, H]

    in_maps = []
    for c in range(NCORES):
        esl = slice(c * E_LOC, (c + 1) * E_LOC)
        ssl = slice(c * SH_LOC, (c + 1) * SH_LOC)
        sel = np.zeros((E_LOC, E, P), dtype=np.float32)
        for el in range(E_LOC):
            sel[el, c * E_LOC + el, :] = 1.0
        # gw/uw: [128, (el, h-tile, i)]
        gw_loc = gwT[esl]            # [E_LOC, H, I]
        gw_p = np.ascontiguousarray(
            gw_loc.reshape(E_LOC, NH, P, I).transpose(2, 0, 1, 3)
            .reshape(P, -1))
        uw_loc = uwT[esl]
        uw_p = np.ascontiguousarray(
            uw_loc.reshape(E_LOC, NH, P, I).transpose(2, 0, 1, 3)
            .reshape(P, -1))
        # dw: [128, (el, i-tile, h)]
        dw_loc = dwT[esl]            # [E_LOC, I, H]
        dw_p = np.ascontiguousarray(
            dw_loc.reshape(E_LOC, NI, P, H).transpose(2, 0, 1, 3)
            .reshape(P, -1))
        in_maps.append({
            "tokf": tokf_p,
            "tokb": tokb_p,
            "rwT": rw_p,
            "rbias": rbias,
            "ident": ident,
            "selb": np.ascontiguousarray(
                sel.transpose(1, 0, 2).reshape(E, -1)).astype(bf),
            "gwT": gw_p,
            "uwT": uw_p,
            "dwT": dw_p,
            "sgwT": _pack_rows(np.ascontiguousarray(sgwT[:, ssl])
                               .astype(bf)),
            "suwT": _pack_rows(np.ascontiguousarray(suwT[:, ssl])
                               .astype(bf)),
            "sdwT": np.ascontiguousarray(sdwT[ssl, :]).astype(bf),
        })
    return in_maps


def run_on_device(inputs: dict, trace: bool = False, tmpdir: str | None = None):
    in_maps = _prep_inputs(**inputs)
    nc = _build(trace=trace)
    res = run_bass_kernel_spmd(nc, in_maps, list(range(NCORES)), trace=trace,
                               tmpdir=tmpdir)
    # Reassemble: for RS piece (a, b) of chunk ck, core c's shard rows
    # [16a, 16a + 16(b-a)) hold global h rows [128a + 16(b-a)c, ...).
    outT = np.empty((H, T), dtype=np.float32)
    for c in range(NCORES):
        sh = np.asarray(res.results[c]["out_shard"], dtype=np.float32)
        for ck in range(NC_):
            csl = slice(ck * TC, (ck + 1) * TC)
            for (a, b) in PIECES[ck]:
                w = (b - a) * P // NCORES
                outT[a * P + c * w:a * P + (c + 1) * w, csl] = \
                    sh[a * P // NCORES:a * P // NCORES + w, csl]
    out = np.ascontiguousarray(outT.T).reshape(B, S, H).astype(np.float32)
    return out, res


def kernel(**inputs) -> np.ndarray:
    out, _ = run_on_device(inputs, trace=False)
    return out
